# revision 21
# baseline (speedup 1.0000x reference)
"""Causal self-attention (B=4, T=2048, C=1024, H=16) on 8 Trainium2 NeuronCores.

Sharding: tensor-parallel over heads. Each core owns 2 heads:
  - Wq/Wk column slices [C, 128] (Wq pre-scaled by 1/sqrt(D)), Wv augmented
    to [C, 130] with two zero columns whose biases are 1.0 (so the "ones"
    denominator column of v_aug comes straight out of the projection),
    Wo row slice [128, C].
  - computes q/k/v for its heads from the full x, flash-style causal
    attention, and a partial output projection (f16); host sums partials.

Per-core schedule (single launch, software-pipelined across batches):
  qT/kT [128(2h x 64d), T] = W.T @ xT   (+bias per-partition via DVE)
  v_aug [128 t, 130]       = xT_tile.T @ Wv_aug + bias_row (DVE add)
  Scores (transposed): psc[j 128, i-chunk<=512] = kT.T @ qT per head,
    heads row-packed at tile_position (0,0)/(64,0) into the two banks of a
    [128, 1024] PSUM tile. Diagonal j-tiles narrowed to the valid i-range
    (widths 512/384/256/128) - 15% less score/exp/PV work.
  e = exp(s - 4) on ACT, one ACTIVATE per step spanning both banks
    (the -4 shift keeps 1/denom in fp16-friendly range; cancels in softmax).
  Causal triangle on the 128-col diagonal edge via gpsimd affine_select.
  py[65, i] += v_aug.T @ e per head (PSUM accumulate over j-tiles); row 64
    is the softmax denominator.
  alpha = reciprocal_approx_fast(denom) (DVE), partition_broadcast (gpsimd),
    yta[128 d2h, i] = py * alpha (DVE, writes both heads into one tile -
    partition-offset DVE write verified on HW).
  out[i 128, c-chunk 512] = yta.T @ Wo, one K=128 matmul, f16 out via DVE.

PE density (HAM warmth): projections of batch b+1 and out-projections of
batch b are emitted as filler work interleaved into attention(b)'s
ACT-bound inner loop; scores are emitted one step ahead of PV so the PE
never sits behind an exp dependency.
"""

import sys

if "/opt/trn_rl_repo" not in sys.path:
    sys.path.insert(0, "/opt/trn_rl_repo")

from collections import deque
from contextlib import ExitStack

import numpy as np

import concourse.bass as bass
import concourse.tile as tile
from concourse import bacc, mybir
from concourse import bass_utils

B, T, C, H, D = 4, 2048, 1024, 16, 64
N_CORES = 8
HPC = H // N_CORES  # heads per core = 2
W = HPC * D  # per-core projection width = 128

F32 = mybir.dt.float32
F16 = mybir.dt.float16
AF = mybir.ActivationFunctionType

ICH = 512  # i (query) chunk in the free dim
NIC = T // ICH  # 4
NKT = C // 128  # 8 contraction tiles for projections
NTT = T // 128  # 16 t-tiles (keys) per batch
M0 = 4.0  # constant score shift inside exp; cancels in softmax

_CACHE = {}


def _kernel_body(ctx, tc, xT, wq, wk, wva, wo, bq, bk, bva, out):
    nc = tc.nc

    const_p = ctx.enter_context(tc.tile_pool(name="const", bufs=1))
    w_p = ctx.enter_context(tc.tile_pool(name="wts", bufs=1))
    xt_p = ctx.enter_context(tc.tile_pool(name="xt", bufs=3 * NKT))
    qk_p = ctx.enter_context(tc.tile_pool(name="qk", bufs=2))
    va_p = ctx.enter_context(tc.tile_pool(name="vaug", bufs=2 * NTT))
    e_p = ctx.enter_context(tc.tile_pool(name="ep", bufs=4))
    yta_p = ctx.enter_context(tc.tile_pool(name="yta", bufs=4))
    r_p = ctx.enter_context(tc.tile_pool(name="rp", bufs=8))
    rb_p = ctx.enter_context(tc.tile_pool(name="rbp", bufs=6))
    ob_p = ctx.enter_context(tc.tile_pool(name="ob", bufs=8))
    psc_p = ctx.enter_context(tc.tile_pool(name="psc", bufs=2, space="PSUM"))
    py_p = ctx.enter_context(tc.tile_pool(name="py", bufs=2, space="PSUM"))
    pm_p = ctx.enter_context(tc.tile_pool(name="pm", bufs=2, space="PSUM"))

    # ---- constants / weights (loaded once; host pre-packs to [128, ...] so
    # each is a single contiguous DMA - the serialized ~600ns DMA triggers on
    # the sync queue were delaying batch-0 xT loads by ~20us) ----
    wq_sb = w_p.tile([128, C], F16, tag="wq")
    wk_sb = w_p.tile([128, C], F16, tag="wk")
    wva_sb = w_p.tile([128, NKT * 130], F16, tag="wva")
    nc.sync.dma_start(wq_sb[:], wq[:])
    nc.sync.dma_start(wk_sb[:], wk[:])
    nc.sync.dma_start(wva_sb[:], wva[:])
    bva_bc = const_p.tile([128, 130], F32, tag="bvab")
    m0t = const_p.tile([128, 1], F32, tag="m0")

    xts = [[None] * NKT for _ in range(B)]
    qTs = [None] * B
    kTs = [None] * B
    vas = [[None] * NTT for _ in range(B)]

    qproj = deque()
    qout = deque()
    state = {"step": 0, "b": 0}
    NSTEPS = sum(4 * ic + 4 for ic in range(NIC))  # 40

    def emit_load(b):
        for kt in range(NKT):
            xt = xt_p.tile([128, T], F16, tag="xt")
            nc.sync.dma_start(xt[:], xT[b, kt * 128 : (kt + 1) * 128, :])
            xts[b][kt] = xt

    def make_proj_thunks(b):
        ths = []

        def alloc(b=b):
            qTs[b] = qk_p.tile([128, T], F16, tag="qT", name="qT")
            kTs[b] = qk_p.tile([128, T], F16, tag="kT", name="kT")

        ths.append(alloc)
        for n in range(NIC):
            for which in ("q", "k"):
                def th(b=b, n=n, which=which):
                    csl = slice(n * ICH, (n + 1) * ICH)
                    wsb = wq_sb if which == "q" else wk_sb
                    bias = bias_q if which == "q" else bias_k
                    dst = qTs[b] if which == "q" else kTs[b]
                    ps = pm_p.tile([128, ICH], F32, tag="pm")
                    for kt in range(NKT):
                        nc.tensor.matmul(
                            ps[:],
                            wsb[:, kt * 128 : (kt + 1) * 128],
                            xts[b][kt][:, csl],
                            start=kt == 0,
                            stop=kt == NKT - 1,
                        )
                    nc.vector.tensor_scalar_add(dst[:, csl], ps[:], bias[:])

                ths.append(th)
        for tt in range(NTT):
            def th(b=b, tt=tt):
                tsl = slice(tt * 128, (tt + 1) * 128)
                ps = pm_p.tile([128, 130], F32, tag="pm")
                for kt in range(NKT):
                    nc.tensor.matmul(
                        ps[:],
                        xts[b][kt][:, tsl],
                        wva_sb[:, kt * 130 : (kt + 1) * 130],
                        start=kt == 0,
                        stop=kt == NKT - 1,
                    )
                va = va_p.tile([128, 130], F16, tag="va")
                nc.vector.tensor_add(va[:], ps[:], bva_bc[:])
                vas[b][tt] = va

            ths.append(th)
        return ths

    def make_outproj_thunks(b, ic, yta):
        ths = []
        for itl in range(4):
            def th(b=b, ic=ic, yta=yta, itl=itl):
                it = ic * 4 + itl
                off = itl * 128
                for nch in range(2):
                    osl = slice(nch * ICH, (nch + 1) * ICH)
                    po = pm_p.tile([128, ICH], F32, tag="pm")
                    nc.tensor.matmul(
                        po[:], yta[:, off : off + 128], wo_sb[:, osl],
                        start=True, stop=True,
                    )
                    obt = ob_p.tile([128, ICH], F16, tag="ob")
                    nc.vector.tensor_copy(obt[:], po[:])
                    nc.sync.dma_start(out[b, it * 128 : (it + 1) * 128, osl], obt[:])

            ths.append(th)
        return ths

    def pace(skip_out=False):
        s = state["step"]
        b = state["b"]
        state["step"] = s + 1
        if qproj:
            if s < NSTEPS - 4:
                k = -(-len(qproj) // (NSTEPS - 4 - s))
            else:
                k = len(qproj)
            for _ in range(min(k, len(qproj))):
                qproj.popleft()()
        # keep the DVE queue shallow around the alpha chain and let the next
        # batch's scores reach the PE FIFO before yta-dependent out-proj MMs.
        # The second-to-last batch hoards out-proj work so the final batch
        # (which has no projection filler) doesn't run dry; the final batch
        # drains eagerly to shorten the tail.
        if b == B - 2 and s > NSTEPS - 12:
            if len(qout) > 8:
                qout.popleft()()
        elif b == B - 1:
            if qout:
                qout.popleft()()
                if len(qout) > 2:
                    qout.popleft()()
        elif qout and (not skip_out or len(qout) > 4):
            qout.popleft()()
            if len(qout) > 6:
                qout.popleft()()

    def emit_alpha_half(py, via_act):
        """denominator -> alpha for one head; the SBUF staging copy is split
        across ACT/DVE so neither queue serializes both heads (custom-DVE
        reciprocal misreads PSUM sources so SBUF staging is required)."""
        dn = r_p.tile([1, ICH], F32, tag="dn")
        if via_act:
            nc.scalar.activation(dn[:], py[64:65, :], AF.Copy)
        else:
            nc.vector.tensor_copy(dn[:], py[64:65, :])
        r = r_p.tile([1, ICH], F32, tag="r")
        nc.vector.reciprocal_approx_fast(r[:], dn[:])
        rb = rb_p.tile([64, ICH], F32, tag="rb")
        nc.gpsimd.partition_broadcast(rb[:], r[:])
        return rb

    def do_batch_attention(b):
        steps = []
        for ic in range(NIC):
            njt = 4 * ic + 4
            i0 = ic * ICH
            for jt in range(njt):
                k = jt - 4 * ic
                if k >= 0:
                    wdt, istart = ICH - 128 * k, 128 * jt
                else:
                    wdt, istart = ICH, i0
                steps.append((ic, jt, njt, i0, wdt, istart))

        pscs = {}

        def emit_scores(si):
            ic, jt, njt, i0, wdt, istart = steps[si]
            # h1 always starts at the second PSUM bank: the row-tiled head MMs
            # run concurrently and must not share a bank
            h1o = ICH
            psc = psc_p.tile([128, 1024], F32, tag="psc")
            jsl = slice(jt * 128, jt * 128 + 128)
            isl = slice(istart, i0 + ICH)
            nc.tensor.matmul(
                psc[:, 0:wdt], kTs[b][0:64, jsl], qTs[b][0:64, isl],
                start=True, stop=True, tile_position=(0, 0),
            )
            nc.tensor.matmul(
                psc[:, h1o : h1o + wdt], kTs[b][64:128, jsl], qTs[b][64:128, isl],
                start=True, stop=True, tile_position=(64, 0),
            )
            pscs[si] = (psc, h1o)

        pys = {}
        emit_scores(0)
        for si in range(len(steps)):
            ic, jt, njt, i0, wdt, istart = steps[si]
            if si + 1 < len(steps):
                emit_scores(si + 1)
            psc, h1o = pscs.pop(si)
            e = e_p.tile([128, 1024], F16, tag="e")
            nc.scalar.activation(
                e[:, 0 : ICH + wdt], psc[:, 0 : ICH + wdt], AF.Exp, bias=m0t[:]
            )
            eh1 = ICH  # h1 slab offset within e
            if jt - 4 * ic >= 0:  # diagonal: zero the j > i triangle (128 cols)
                for off in (0, eh1):
                    nc.gpsimd.affine_select(
                        out=e[:, off : off + 128],
                        in_=e[:, off : off + 128],
                        pattern=[[1, 128]],
                        compare_op=mybir.AluOpType.is_ge,
                        fill=0.0,
                        base=0,
                        channel_multiplier=-1,
                    )
            if jt == 0:
                pys[ic] = (
                    py_p.tile([65, ICH], F32, tag="py", name="py0"),
                    py_p.tile([65, ICH], F32, tag="py", name="py1"),
                )
            py0, py1 = pys[ic]
            coff = istart - i0
            st, sp = jt == 0, jt == njt - 1

            nc.tensor.matmul(
                py0[:, coff:ICH], vas[b][jt][:, 0:65], e[:, 0:wdt], start=st, stop=sp
            )
            if sp:
                rb0 = emit_alpha_half(py0, via_act=True)
            nc.tensor.matmul(
                py1[:, coff:ICH], vas[b][jt][:, 65:130], e[:, eh1 : eh1 + wdt],
                start=st, stop=sp,
            )
            if sp:
                rb1 = emit_alpha_half(py1, via_act=False)
                yta = yta_p.tile([128, ICH], F16, tag="yta")
                nc.vector.tensor_mul(yta[0:64, :], py0[0:64, :], rb0[:])
                nc.vector.tensor_mul(yta[64:128, :], py1[0:64, :], rb1[:])
                pys.pop(ic)
                qout.extend(make_outproj_thunks(b, ic, yta))
            pace(skip_out=sp or si <= 1)

    # ---- pipeline over batches ----
    # tiny bias loads go before the bulk xT transfers: the first DVE
    # bias-add (and with it the whole PE pipeline) gates on them
    bias_q = const_p.tile([W, 1], F32, tag="bq")
    bias_k = const_p.tile([W, 1], F32, tag="bk")
    nc.sync.dma_start(bias_q[:], bq[:])
    nc.sync.dma_start(bias_k[:], bk[:])
    bva_row = const_p.tile([1, 130], F32, tag="bvar")
    nc.sync.dma_start(bva_row[:], bva[:])
    nc.gpsimd.partition_broadcast(bva_bc[:], bva_row[:])
    nc.gpsimd.memset(m0t[:], -M0)
    emit_load(0)
    emit_load(1)
    wo_sb = w_p.tile([128, C], F16, tag="wo")
    nc.sync.dma_start(wo_sb[:], wo[:])
    # minimal prologue: chunk-0 q/k and the first 4 v-tiles unblock
    # attention(0) ic0; the rest of proj(0) drains as filler inside
    # attention(0), in dependency order ahead of each ic's needs
    L = make_proj_thunks(0)
    alloc0, qk0, v0 = L[0], L[1:9], L[9:25]
    alloc0()
    for th in (qk0[0], qk0[1], *v0[0:4]):
        th()
    rest0 = []
    for n in (1, 2, 3):
        rest0 += [qk0[2 * n], qk0[2 * n + 1]]
        rest0 += v0[4 * n : 4 * n + 4]
    qproj.extend(rest0)
    for b in range(B):
        if b + 2 < B:
            emit_load(b + 2)
        if b + 1 < B:
            qproj.extend(make_proj_thunks(b + 1))
        state["step"] = 0
        state["b"] = b
        do_batch_attention(b)
        while qproj:  # proj(b+1) must be complete before attention(b+1)
            qproj.popleft()()
    while qout:
        qout.popleft()()


def _build():
    if "nc" in _CACHE:
        return _CACHE["nc"]
    nc = bacc.Bacc("TRN2", target_bir_lowering=False, debug=False, num_devices=N_CORES)
    xT = nc.dram_tensor("xT", [B, C, T], F16, kind="ExternalInput").ap()
    wq = nc.dram_tensor("wq", [128, C], F16, kind="ExternalInput").ap()
    wk = nc.dram_tensor("wk", [128, C], F16, kind="ExternalInput").ap()
    wva = nc.dram_tensor("wva", [128, NKT * 130], F16, kind="ExternalInput").ap()
    wo = nc.dram_tensor("wo", [W, C], F16, kind="ExternalInput").ap()
    bq = nc.dram_tensor("bq", [W, 1], F32, kind="ExternalInput").ap()
    bk = nc.dram_tensor("bk", [W, 1], F32, kind="ExternalInput").ap()
    bva = nc.dram_tensor("bva", [1, 130], F32, kind="ExternalInput").ap()
    out = nc.dram_tensor("out", [B, T, C], F16, kind="ExternalOutput").ap()

    with tile.TileContext(nc) as tc:
        with ExitStack() as ctx:
            _kernel_body(ctx, tc, xT, wq, wk, wva, wo, bq, bk, bva, out)
    nc.compile()
    _CACHE["nc"] = nc
    return nc


def make_in_maps(inputs):
    x = np.asarray(inputs["x"], np.float32)
    Wq = np.asarray(inputs["Wq"], np.float32)
    bq = np.asarray(inputs["bq"], np.float32)
    Wk = np.asarray(inputs["Wk"], np.float32)
    bk = np.asarray(inputs["bk"], np.float32)
    Wv = np.asarray(inputs["Wv"], np.float32)
    bv = np.asarray(inputs["bv"], np.float32)
    Wo = np.asarray(inputs["Wo"], np.float32)

    scale = np.float32(1.0 / np.sqrt(D))
    xT = np.ascontiguousarray(x.transpose(0, 2, 1)).astype(np.float16)  # [B, C, T]
    Wq_s = Wq * scale
    bq_s = bq * scale

    in_maps = []
    for c in range(N_CORES):
        s = slice(c * W, (c + 1) * W)
        wva = np.zeros((C, 130), np.float32)
        wva[:, 0:64] = Wv[:, c * W : c * W + D]
        wva[:, 65:129] = Wv[:, c * W + D : (c + 1) * W]
        wva_p = wva.reshape(NKT, 128, 130).transpose(1, 0, 2).reshape(128, NKT * 130)
        wq_p = (
            np.ascontiguousarray(Wq_s[:, s])
            .reshape(NKT, 128, W)
            .transpose(1, 0, 2)
            .reshape(128, C)
        )
        wk_p = (
            np.ascontiguousarray(Wk[:, s])
            .reshape(NKT, 128, W)
            .transpose(1, 0, 2)
            .reshape(128, C)
        )
        bva = np.zeros((1, 130), np.float32)
        bva[0, 0:64] = bv[c * W : c * W + D]
        bva[0, 64] = 1.0
        bva[0, 65:129] = bv[c * W + D : (c + 1) * W]
        bva[0, 129] = 1.0
        in_maps.append(
            {
                "xT": xT,
                "wq": np.ascontiguousarray(wq_p).astype(np.float16),
                "wk": np.ascontiguousarray(wk_p).astype(np.float16),
                "wva": np.ascontiguousarray(wva_p).astype(np.float16),
                "wo": np.ascontiguousarray(Wo[s, :]).astype(np.float16),
                "bq": np.ascontiguousarray(bq_s[s, None]),
                "bk": np.ascontiguousarray(bk[s, None]),
                "bva": bva,
            }
        )
    return in_maps


def kernel(**inputs):
    nc = _build()
    in_maps = make_in_maps(inputs)
    res = bass_utils.run_bass_kernel_spmd(nc, in_maps, core_ids=list(range(N_CORES)))
    bo = np.asarray(inputs["bo"], np.float32)
    out = np.zeros((B, T, C), np.float32)
    for c in range(N_CORES):
        out += res.results[c]["out"].astype(np.float32)
    out += bo
    return out


if __name__ == "__main__":
    rng = np.random.default_rng(0)
    ins = {
        "x": rng.standard_normal((B, T, C), dtype=np.float32),
        "Wq": rng.standard_normal((C, C), dtype=np.float32) / 32,
        "bq": rng.standard_normal((C,), dtype=np.float32) * 0.02,
        "Wk": rng.standard_normal((C, C), dtype=np.float32) / 32,
        "bk": rng.standard_normal((C,), dtype=np.float32) * 0.02,
        "Wv": rng.standard_normal((C, C), dtype=np.float32) / 32,
        "bv": rng.standard_normal((C,), dtype=np.float32) * 0.02,
        "Wo": rng.standard_normal((C, C), dtype=np.float32) / 32,
        "bo": rng.standard_normal((C,), dtype=np.float32) * 0.02,
    }
    got = kernel(**ins)
    print("kernel ran, out shape", got.shape)


# revision 22
# speedup vs baseline: 1.0225x; 1.0225x over previous
"""Causal self-attention (B=4, T=2048, C=1024, H=16) on 8 Trainium2 NeuronCores.

Sharding: tensor-parallel over heads. Each core owns 2 heads:
  - Wq/Wk column slices [C, 128] (Wq pre-scaled by 1/sqrt(D)), Wv augmented
    to [C, 130] with two zero columns whose biases are 1.0 (so the "ones"
    denominator column of v_aug comes straight out of the projection),
    Wo row slice [128, C].
  - computes q/k/v for its heads from the full x, flash-style causal
    attention, and a partial output projection (f16); host sums partials.

Per-core schedule (single launch, software-pipelined across batches):
  qT/kT [128(2h x 64d), T] = W.T @ xT   (+bias per-partition via DVE)
  v_aug [128 t, 130]       = xT_tile.T @ Wv_aug + bias_row (DVE add)
  Scores (transposed): psc[j 128, i-chunk<=512] = kT.T @ qT per head,
    heads row-packed at tile_position (0,0)/(64,0) into the two banks of a
    [128, 1024] PSUM tile. Diagonal j-tiles narrowed to the valid i-range
    (widths 512/384/256/128) - 15% less score/exp/PV work.
  e = exp(s - 4) on ACT, one ACTIVATE per step spanning both banks
    (the -4 shift keeps 1/denom in fp16-friendly range; cancels in softmax).
  Causal triangle on the 128-col diagonal edge via gpsimd affine_select.
  py[65, i] += v_aug.T @ e per head (PSUM accumulate over j-tiles); row 64
    is the softmax denominator.
  alpha = reciprocal_approx_fast(denom) (DVE), partition_broadcast (gpsimd),
    yta[128 d2h, i] = py * alpha (DVE, writes both heads into one tile -
    partition-offset DVE write verified on HW).
  out[i 128, c-chunk 512] = yta.T @ Wo, one K=128 matmul, f16 out via DVE.

PE density (HAM warmth): projections of batch b+1 and out-projections of
batch b are emitted as filler work interleaved into attention(b)'s
ACT-bound inner loop; scores are emitted one step ahead of PV so the PE
never sits behind an exp dependency.
"""

import sys

if "/opt/trn_rl_repo" not in sys.path:
    sys.path.insert(0, "/opt/trn_rl_repo")

from collections import deque
from contextlib import ExitStack

import numpy as np

import concourse.bass as bass
import concourse.tile as tile
from concourse import bacc, mybir
from concourse import bass_utils

B, T, C, H, D = 4, 2048, 1024, 16, 64
N_CORES = 8
HPC = H // N_CORES  # heads per core = 2
W = HPC * D  # per-core projection width = 128

F32 = mybir.dt.float32
F16 = mybir.dt.float16
AF = mybir.ActivationFunctionType

ICH = 512  # i (query) chunk in the free dim
NIC = T // ICH  # 4
NKT = C // 128  # 8 contraction tiles for projections
NTT = T // 128  # 16 t-tiles (keys) per batch
M0 = 4.0  # constant score shift inside exp; cancels in softmax

_CACHE = {}


def _kernel_body(ctx, tc, xT, wq, wk, wva, wo, bq, bk, bva, out):
    nc = tc.nc

    const_p = ctx.enter_context(tc.tile_pool(name="const", bufs=1))
    w_p = ctx.enter_context(tc.tile_pool(name="wts", bufs=1))
    xt_p = ctx.enter_context(tc.tile_pool(name="xt", bufs=3 * NKT))
    qk_p = ctx.enter_context(tc.tile_pool(name="qk", bufs=2))
    va_p = ctx.enter_context(tc.tile_pool(name="vaug", bufs=2 * NTT))
    e_p = ctx.enter_context(tc.tile_pool(name="ep", bufs=4))
    yta_p = ctx.enter_context(tc.tile_pool(name="yta", bufs=4))
    r_p = ctx.enter_context(tc.tile_pool(name="rp", bufs=8))
    rb_p = ctx.enter_context(tc.tile_pool(name="rbp", bufs=6))
    ob_p = ctx.enter_context(tc.tile_pool(name="ob", bufs=8))
    psc_p = ctx.enter_context(tc.tile_pool(name="psc", bufs=2, space="PSUM"))
    py_p = ctx.enter_context(tc.tile_pool(name="py", bufs=2, space="PSUM"))
    pm_p = ctx.enter_context(tc.tile_pool(name="pm", bufs=2, space="PSUM"))

    # ---- constants / weights (loaded once; host pre-packs to [128, ...] so
    # each is a single contiguous DMA - the serialized ~600ns DMA triggers on
    # the sync queue were delaying batch-0 xT loads by ~20us) ----
    wq_sb = w_p.tile([128, C], F16, tag="wq")
    wk_sb = w_p.tile([128, C], F16, tag="wk")
    wva_sb = w_p.tile([128, NKT * 130], F16, tag="wva")
    nc.sync.dma_start(wq_sb[:], wq[:])
    nc.sync.dma_start(wk_sb[:], wk[:])
    nc.sync.dma_start(wva_sb[:], wva[:])
    bva_bc = const_p.tile([128, 130], F32, tag="bvab")
    m0t = const_p.tile([128, 1], F32, tag="m0")

    xts = [[None] * NKT for _ in range(B)]
    qTs = [None] * B
    kTs = [None] * B
    vas = [[None] * NTT for _ in range(B)]

    qproj = deque()
    qout = deque()
    state = {"step": 0, "b": 0}
    NSTEPS = sum(4 * ic + 4 for ic in range(NIC))  # 40

    def emit_load(b):
        for kt in range(NKT):
            xt = xt_p.tile([128, T], F16, tag="xt")
            nc.sync.dma_start(xt[:], xT[b, kt * 128 : (kt + 1) * 128, :])
            xts[b][kt] = xt

    def make_proj_thunks(b):
        ths = []

        def alloc(b=b):
            qTs[b] = qk_p.tile([128, T], F16, tag="qT", name="qT")
            kTs[b] = qk_p.tile([128, T], F16, tag="kT", name="kT")

        ths.append(alloc)
        for n in range(NIC):
            for which in ("q", "k"):
                def th(b=b, n=n, which=which):
                    csl = slice(n * ICH, (n + 1) * ICH)
                    wsb = wq_sb if which == "q" else wk_sb
                    bias = bias_q if which == "q" else bias_k
                    dst = qTs[b] if which == "q" else kTs[b]
                    ps = pm_p.tile([128, ICH], F32, tag="pm")
                    for kt in range(NKT):
                        nc.tensor.matmul(
                            ps[:],
                            wsb[:, kt * 128 : (kt + 1) * 128],
                            xts[b][kt][:, csl],
                            start=kt == 0,
                            stop=kt == NKT - 1,
                        )
                    nc.vector.tensor_scalar_add(dst[:, csl], ps[:], bias[:])

                ths.append(th)
        for tt in range(NTT):
            def th(b=b, tt=tt):
                tsl = slice(tt * 128, (tt + 1) * 128)
                ps = pm_p.tile([128, 130], F32, tag="pm")
                for kt in range(NKT):
                    nc.tensor.matmul(
                        ps[:],
                        xts[b][kt][:, tsl],
                        wva_sb[:, kt * 130 : (kt + 1) * 130],
                        start=kt == 0,
                        stop=kt == NKT - 1,
                    )
                va = va_p.tile([128, 130], F16, tag="va")
                nc.vector.tensor_add(va[:], ps[:], bva_bc[:])
                vas[b][tt] = va

            ths.append(th)
        return ths

    def make_outproj_thunks(b, ic, yta):
        ths = []
        for itl in range(4):
            def th(b=b, ic=ic, yta=yta, itl=itl):
                it = ic * 4 + itl
                off = itl * 128
                for nch in range(2):
                    osl = slice(nch * ICH, (nch + 1) * ICH)
                    po = pm_p.tile([128, ICH], F32, tag="pm")
                    nc.tensor.matmul(
                        po[:], yta[:, off : off + 128], wo_sb[:, osl],
                        start=True, stop=True,
                    )
                    obt = ob_p.tile([128, ICH], F16, tag="ob")
                    nc.vector.tensor_copy(obt[:], po[:])
                    nc.sync.dma_start(out[b, it * 128 : (it + 1) * 128, osl], obt[:])

            ths.append(th)
        return ths

    def pace(skip_out=False):
        s = state["step"]
        state["step"] = s + 1
        if qproj:
            if s < NSTEPS - 4:
                k = -(-len(qproj) // (NSTEPS - 4 - s))
            else:
                k = len(qproj)
            for _ in range(min(k, len(qproj))):
                qproj.popleft()()
        # keep the DVE queue shallow around the alpha chain and let the next
        # batch's scores reach the PE FIFO before yta-dependent out-proj MMs
        if qout and (not skip_out or len(qout) > 4):
            qout.popleft()()
            if len(qout) > 6:
                qout.popleft()()

    def emit_alpha_half(py, via_act):
        """denominator -> alpha for one head; the SBUF staging copy is split
        across ACT/DVE so neither queue serializes both heads (custom-DVE
        reciprocal misreads PSUM sources so SBUF staging is required)."""
        dn = r_p.tile([1, ICH], F32, tag="dn")
        if via_act:
            nc.scalar.activation(dn[:], py[64:65, :], AF.Copy)
        else:
            nc.vector.tensor_copy(dn[:], py[64:65, :])
        r = r_p.tile([1, ICH], F32, tag="r")
        nc.vector.reciprocal_approx_fast(r[:], dn[:])
        rb = rb_p.tile([64, ICH], F32, tag="rb")
        nc.gpsimd.partition_broadcast(rb[:], r[:])
        return rb

    def do_batch_attention(b):
        steps = []
        for ic in range(NIC):
            njt = 4 * ic + 4
            i0 = ic * ICH
            for jt in range(njt):
                k = jt - 4 * ic
                if k >= 0:
                    wdt, istart = ICH - 128 * k, 128 * jt
                else:
                    wdt, istart = ICH, i0
                steps.append((ic, jt, njt, i0, wdt, istart))

        pscs = {}

        def emit_scores(si):
            ic, jt, njt, i0, wdt, istart = steps[si]
            # h1 always starts at the second PSUM bank: the row-tiled head MMs
            # run concurrently and must not share a bank
            h1o = ICH
            psc = psc_p.tile([128, 1024], F32, tag="psc")
            jsl = slice(jt * 128, jt * 128 + 128)
            isl = slice(istart, i0 + ICH)
            nc.tensor.matmul(
                psc[:, 0:wdt], kTs[b][0:64, jsl], qTs[b][0:64, isl],
                start=True, stop=True, tile_position=(0, 0),
            )
            nc.tensor.matmul(
                psc[:, h1o : h1o + wdt], kTs[b][64:128, jsl], qTs[b][64:128, isl],
                start=True, stop=True, tile_position=(64, 0),
            )
            pscs[si] = (psc, h1o)

        pys = {}
        emit_scores(0)
        for si in range(len(steps)):
            ic, jt, njt, i0, wdt, istart = steps[si]
            if si + 1 < len(steps):
                emit_scores(si + 1)
            psc, h1o = pscs.pop(si)
            e = e_p.tile([128, 1024], F16, tag="e")
            nc.scalar.activation(
                e[:, 0 : ICH + wdt], psc[:, 0 : ICH + wdt], AF.Exp, bias=m0t[:]
            )
            eh1 = ICH  # h1 slab offset within e
            if jt - 4 * ic >= 0:  # diagonal: zero the j > i triangle (128 cols)
                for off in (0, eh1):
                    nc.gpsimd.affine_select(
                        out=e[:, off : off + 128],
                        in_=e[:, off : off + 128],
                        pattern=[[1, 128]],
                        compare_op=mybir.AluOpType.is_ge,
                        fill=0.0,
                        base=0,
                        channel_multiplier=-1,
                    )
            if jt == 0:
                pys[ic] = (
                    py_p.tile([65, ICH], F32, tag="py", name="py0"),
                    py_p.tile([65, ICH], F32, tag="py", name="py1"),
                )
            py0, py1 = pys[ic]
            coff = istart - i0
            st, sp = jt == 0, jt == njt - 1

            nc.tensor.matmul(
                py0[:, coff:ICH], vas[b][jt][:, 0:65], e[:, 0:wdt], start=st, stop=sp
            )
            if sp:
                rb0 = emit_alpha_half(py0, via_act=True)
            nc.tensor.matmul(
                py1[:, coff:ICH], vas[b][jt][:, 65:130], e[:, eh1 : eh1 + wdt],
                start=st, stop=sp,
            )
            if sp:
                rb1 = emit_alpha_half(py1, via_act=False)
                yta = yta_p.tile([128, ICH], F16, tag="yta")
                nc.vector.tensor_mul(yta[0:64, :], py0[0:64, :], rb0[:])
                nc.vector.tensor_mul(yta[64:128, :], py1[0:64, :], rb1[:])
                pys.pop(ic)
                qout.extend(make_outproj_thunks(b, ic, yta))
            pace(skip_out=sp or si <= 1)

    # ---- pipeline over batches ----
    # tiny bias loads go before the bulk xT transfers: the first DVE
    # bias-add (and with it the whole PE pipeline) gates on them
    bias_q = const_p.tile([W, 1], F32, tag="bq")
    bias_k = const_p.tile([W, 1], F32, tag="bk")
    nc.sync.dma_start(bias_q[:], bq[:])
    nc.sync.dma_start(bias_k[:], bk[:])
    bva_row = const_p.tile([1, 130], F32, tag="bvar")
    nc.sync.dma_start(bva_row[:], bva[:])
    nc.gpsimd.partition_broadcast(bva_bc[:], bva_row[:])
    nc.gpsimd.memset(m0t[:], -M0)
    emit_load(0)
    emit_load(1)
    wo_sb = w_p.tile([128, C], F16, tag="wo")
    nc.sync.dma_start(wo_sb[:], wo[:])
    # minimal prologue: chunk-0 q/k and the first 4 v-tiles unblock
    # attention(0) ic0; the rest of proj(0) drains as filler inside
    # attention(0), in dependency order ahead of each ic's needs
    L = make_proj_thunks(0)
    alloc0, qk0, v0 = L[0], L[1:9], L[9:25]
    alloc0()
    for th in (qk0[0], qk0[1], *v0[0:4]):
        th()
    rest0 = []
    for n in (1, 2, 3):
        rest0 += [qk0[2 * n], qk0[2 * n + 1]]
        rest0 += v0[4 * n : 4 * n + 4]
    qproj.extend(rest0)
    for b in range(B):
        if b + 2 < B:
            emit_load(b + 2)
        if b + 1 < B:
            qproj.extend(make_proj_thunks(b + 1))
        state["step"] = 0
        state["b"] = b
        do_batch_attention(b)
        while qproj:  # proj(b+1) must be complete before attention(b+1)
            qproj.popleft()()
    while qout:
        qout.popleft()()


def _build():
    if "nc" in _CACHE:
        return _CACHE["nc"]
    nc = bacc.Bacc("TRN2", target_bir_lowering=False, debug=False, num_devices=N_CORES)
    xT = nc.dram_tensor("xT", [B, C, T], F16, kind="ExternalInput").ap()
    wq = nc.dram_tensor("wq", [128, C], F16, kind="ExternalInput").ap()
    wk = nc.dram_tensor("wk", [128, C], F16, kind="ExternalInput").ap()
    wva = nc.dram_tensor("wva", [128, NKT * 130], F16, kind="ExternalInput").ap()
    wo = nc.dram_tensor("wo", [W, C], F16, kind="ExternalInput").ap()
    bq = nc.dram_tensor("bq", [W, 1], F32, kind="ExternalInput").ap()
    bk = nc.dram_tensor("bk", [W, 1], F32, kind="ExternalInput").ap()
    bva = nc.dram_tensor("bva", [1, 130], F32, kind="ExternalInput").ap()
    out = nc.dram_tensor("out", [B, T, C], F16, kind="ExternalOutput").ap()

    with tile.TileContext(nc) as tc:
        with ExitStack() as ctx:
            _kernel_body(ctx, tc, xT, wq, wk, wva, wo, bq, bk, bva, out)
    nc.compile()
    _CACHE["nc"] = nc
    return nc


def make_in_maps(inputs):
    x = np.asarray(inputs["x"], np.float32)
    Wq = np.asarray(inputs["Wq"], np.float32)
    bq = np.asarray(inputs["bq"], np.float32)
    Wk = np.asarray(inputs["Wk"], np.float32)
    bk = np.asarray(inputs["bk"], np.float32)
    Wv = np.asarray(inputs["Wv"], np.float32)
    bv = np.asarray(inputs["bv"], np.float32)
    Wo = np.asarray(inputs["Wo"], np.float32)

    scale = np.float32(1.0 / np.sqrt(D))
    xT = np.ascontiguousarray(x.transpose(0, 2, 1)).astype(np.float16)  # [B, C, T]
    Wq_s = Wq * scale
    bq_s = bq * scale

    in_maps = []
    for c in range(N_CORES):
        s = slice(c * W, (c + 1) * W)
        wva = np.zeros((C, 130), np.float32)
        wva[:, 0:64] = Wv[:, c * W : c * W + D]
        wva[:, 65:129] = Wv[:, c * W + D : (c + 1) * W]
        wva_p = wva.reshape(NKT, 128, 130).transpose(1, 0, 2).reshape(128, NKT * 130)
        wq_p = (
            np.ascontiguousarray(Wq_s[:, s])
            .reshape(NKT, 128, W)
            .transpose(1, 0, 2)
            .reshape(128, C)
        )
        wk_p = (
            np.ascontiguousarray(Wk[:, s])
            .reshape(NKT, 128, W)
            .transpose(1, 0, 2)
            .reshape(128, C)
        )
        bva = np.zeros((1, 130), np.float32)
        bva[0, 0:64] = bv[c * W : c * W + D]
        bva[0, 64] = 1.0
        bva[0, 65:129] = bv[c * W + D : (c + 1) * W]
        bva[0, 129] = 1.0
        in_maps.append(
            {
                "xT": xT,
                "wq": np.ascontiguousarray(wq_p).astype(np.float16),
                "wk": np.ascontiguousarray(wk_p).astype(np.float16),
                "wva": np.ascontiguousarray(wva_p).astype(np.float16),
                "wo": np.ascontiguousarray(Wo[s, :]).astype(np.float16),
                "bq": np.ascontiguousarray(bq_s[s, None]),
                "bk": np.ascontiguousarray(bk[s, None]),
                "bva": bva,
            }
        )
    return in_maps


def kernel(**inputs):
    nc = _build()
    in_maps = make_in_maps(inputs)
    res = bass_utils.run_bass_kernel_spmd(nc, in_maps, core_ids=list(range(N_CORES)))
    bo = np.asarray(inputs["bo"], np.float32)
    out = np.zeros((B, T, C), np.float32)
    for c in range(N_CORES):
        out += res.results[c]["out"].astype(np.float32)
    out += bo
    return out


if __name__ == "__main__":
    rng = np.random.default_rng(0)
    ins = {
        "x": rng.standard_normal((B, T, C), dtype=np.float32),
        "Wq": rng.standard_normal((C, C), dtype=np.float32) / 32,
        "bq": rng.standard_normal((C,), dtype=np.float32) * 0.02,
        "Wk": rng.standard_normal((C, C), dtype=np.float32) / 32,
        "bk": rng.standard_normal((C,), dtype=np.float32) * 0.02,
        "Wv": rng.standard_normal((C, C), dtype=np.float32) / 32,
        "bv": rng.standard_normal((C,), dtype=np.float32) * 0.02,
        "Wo": rng.standard_normal((C, C), dtype=np.float32) / 32,
        "bo": rng.standard_normal((C,), dtype=np.float32) * 0.02,
    }
    got = kernel(**ins)
    print("kernel ran, out shape", got.shape)
